# revision 14
# baseline (speedup 1.0000x reference)
"""Trainium2 Bass kernel for nn_MixChan (dense_mlp).

Reference computation (per batch sample b):
    d   = dist / dist.sum()                       # (32,)
    xs  = x.sum(axis=K) * d[c]                    # (32, 512, 512)
    ds  = avgpool4x4(xs)                          # (32, 128, 128)
    h1  = leaky_relu(ds.flat @ W1.T + b1, 0.2)    # (32, 1024)
    coef= leaky_relu(h1 @ W2.T + b2, 0.2)         # (32, 1)
    out = einsum('c,cwh->wh', coef, xs) / 32      # (512, 512)

Sharding: data-parallel over batch B=8 across the 8 NeuronCores; the MLP
weights are replicated (W1 is streamed from HBM as bf16).

x is repacked on the host (pure layout permutation, no arithmetic) into
[(g2), (wl c), (j k h)] so that every device-side DMA is a fully linear
[128, N] transfer: contiguous per partition, balanced across all 16 SDMA
engines, and cheap to issue from the HWDGE (the transposed-AP DMA of the
previous version cost ~5.7us of descriptor generation per group and
landed on only 4 SDMA engines).

Per-core dataflow (one pass over x, fused):
  for each pair gp of row groups (64 pairs; group g = 4 image rows):
    - DMA x pair slab [128p=(wl,c), 2048=(j,k,h)] fp32, 1 MiB linear
    - every 4th group: DMA W1T quad [128, 4096] bf16 (2 MiB linear,
      issued on the scalar HWDGE ring to split issue load)
    per group g in the pair:
    - DVE: k-sum -> xs slab slice [128, 512] bf16 (kept in SBUF, 16 MiB)
    - DVE: reduce over h-groups of 4 -> red4 [128, 128] bf16
    - PE : red4.T @ A -> pooledT [128f, 32c] (A folds the w-sum, the /16
           avgpool normalization and the per-channel dist weight d_c)
    - ACT: copy/cast pooledT -> bf16
    - PE : h1 += pooledT.T @ W1T chunk (PSUM accumulate over 128 chunks)
  MLP tail (tiny), building S [128, 4] = u_c * delta(w, m), u = coef*d/32
  for each quarter q (32 groups):
    - PE : out rows = S.T @ xs slab slice  (the weighted channel sum)
    - ACT/DVE: copy PSUM -> staging slab rows
    - DMA staging slab [128, 512] -> out rows (256 KiB linear)
"""

import numpy as np
import ml_dtypes

B, C, K, W = 8, 32, 2, 512
P = 4                 # pool kernel/stride
G = W // P            # 128 row groups
FEAT = G * G          # 16384
HID = 1024
NEG = 0.2
N_CORES = 8

_prog_cache = {}


def _build_program():
    import concourse.bass as bass
    import concourse.tile as tile
    from concourse import bacc, mybir

    dt = mybir.dt
    f32 = dt.float32
    bf16 = dt.bfloat16
    Alu = mybir.AluOpType

    nc = bacc.Bacc(
        "TRN2",
        debug=False,
        enable_asserts=False,
        target_bir_lowering=False,
        num_devices=N_CORES,
    )

    # x repacked: [gq, (wl c), (j k h)]  (see prep_in_maps)
    x_t = nc.dram_tensor("x", [G // 4, 128, 4 * K * W], f32, kind="ExternalInput").ap()
    # W1T reordered in quads: [gq, p, (i col)] = W1.T[512*gq + 128*i + p, col]
    w1q_t = nc.dram_tensor("w1q", [G // 4, 128, 4 * HID], bf16, kind="ExternalInput").ap()
    amat_t = nc.dram_tensor("amat", [128, C], bf16, kind="ExternalInput").ap()
    # m0big[(wl c), 36j + wl] = 1: mask so that matmul j of an 8-group batch
    # scatters its 4 output rows to partitions 4j..4j+3 of a [32, 512] bank
    m0_t = nc.dram_tensor("m0", [128, 8 * C], bf16, kind="ExternalInput").ap()
    t128_t = nc.dram_tensor("t128", [128, 128], f32, kind="ExternalInput").ap()
    b1r_t = nc.dram_tensor("b1r", [C, HID], f32, kind="ExternalInput").ap()
    w2r_t = nc.dram_tensor("w2r", [C, HID], f32, kind="ExternalInput").ap()
    b2r_t = nc.dram_tensor("b2r", [C, 1], f32, kind="ExternalInput").ap()
    out_t = nc.dram_tensor("out", [W, W], f32, kind="ExternalOutput").ap()

    with tile.TileContext(nc) as tc:
        with (
            tc.tile_pool(name="singles", bufs=1) as singles,
            tc.tile_pool(name="small", bufs=1) as small,
        ):
            # constants (allocated here; DMAs issued after the first x/W1
            # quads so the big streams start immediately)
            amat_sb = singles.tile([128, C], bf16)
            m0_sb = singles.tile([128, 8 * C], bf16)
            t128_sb = singles.tile([128, 128], f32)
            b1r_sb = singles.tile([C, HID], f32)
            w2r_sb = singles.tile([C, HID], f32)
            b2r_sb = singles.tile([C, 1], f32)

            # xs slab: k-summed (unscaled) x, bf16, [(w4 c32), (g128 h512)]
            xs_sb = singles.tile([128, G * W], bf16)
            S_sb = singles.tile([128, 8 * C], bf16)

            with tc.tile_pool(name="psum_h1", bufs=1, space="PSUM") as ph1:
                h1p = ph1.tile([C, HID], f32)

                with (
                    tc.tile_pool(name="xt", bufs=2) as xtp,
                    tc.tile_pool(name="w1c", bufs=2) as w1p,
                    tc.tile_pool(name="red", bufs=2) as redp,
                    tc.tile_pool(name="ptsb", bufs=2) as ptp,
                    tc.tile_pool(name="psum_pt", bufs=2, space="PSUM") as pptp,
                ):
                    for gq in range(G // 4):
                        xt = xtp.tile([128, 4 * K * W], f32)
                        nc.sync.dma_start(xt[:], x_t[gq])
                        w1c = w1p.tile([128, 4 * HID], bf16)
                        nc.scalar.dma_start(w1c[:], w1q_t[gq])
                        if gq == 0:
                            nc.scalar.dma_start(amat_sb[:], amat_t)
                            nc.sync.dma_start(m0_sb[:], m0_t)
                            nc.sync.dma_start(t128_sb[:], t128_t)
                            nc.sync.dma_start(b1r_sb[:], b1r_t)
                            nc.sync.dma_start(w2r_sb[:], w2r_t)
                            nc.sync.dma_start(b2r_sb[:], b2r_t)

                        for j in range(4):
                            g = 4 * gq + j
                            i = g % 4

                            xs_slice = xs_sb[:, W * g : W * (g + 1)]
                            nc.vector.tensor_add(
                                xs_slice,
                                xt[:, 1024 * j : 1024 * j + 512],
                                xt[:, 1024 * j + 512 : 1024 * j + 1024],
                            )

                            red4 = redp.tile([128, G], bf16)
                            with nc.allow_low_precision(
                                reason="4-elem sums feeding a bf16 matmul"
                            ):
                                nc.vector.reduce_sum(
                                    red4[:],
                                    xs_slice.rearrange("p (j f) -> p j f", f=P),
                                    axis=mybir.AxisListType.X,
                                )

                            ptps = pptp.tile([128, C], f32)
                            nc.tensor.matmul(
                                ptps[:], lhsT=red4[:], rhs=amat_sb[:],
                                start=True, stop=True,
                            )
                            ptsb = ptp.tile([128, C], bf16)
                            nc.scalar.copy(ptsb[:], ptps[:])

                            first, last = g == 0, g == G - 1
                            nc.tensor.matmul(
                                h1p[:, 0:512],
                                lhsT=ptsb[:],
                                rhs=w1c[:, 1024 * i : 1024 * i + 512],
                                start=first, stop=last,
                            )
                            nc.tensor.matmul(
                                h1p[:, 512:1024],
                                lhsT=ptsb[:],
                                rhs=w1c[:, 1024 * i + 512 : 1024 * i + 1024],
                                start=first, stop=last,
                            )

                # ---- PE pre-warm: keep TensorE busy through the DVE-only
                # MLP tail so the stage-2 matmul burst starts at the warm
                # (unthrottled) clock instead of paying the HAM ramp.
                with tc.tile_pool(name="psum_w", bufs=1, space="PSUM") as pwp:
                    warm = pwp.tile([128, 128], f32)
                    for _ in range(40):
                        nc.tensor.matmul(
                            warm[:], lhsT=m0_sb[:, 0:128], rhs=m0_sb[:, 0:128],
                            start=True, stop=True,
                        )

                # ---- MLP tail ----
                tmp1 = small.tile([C, HID], f32)
                nc.vector.tensor_add(tmp1[:], h1p[:], b1r_sb[:])
                h1s = small.tile([C, HID], f32)
                nc.vector.scalar_tensor_tensor(
                    out=h1s[:], in0=tmp1[:], scalar=NEG, in1=tmp1[:],
                    op0=Alu.mult, op1=Alu.max,
                )
                # coef = leaky(h1s @ W2.T + b2): tensor_tensor_reduce crashes
                # the exec unit on HW, so mul + reduce + add instead.
                prod = small.tile([C, HID], f32)
                nc.vector.tensor_mul(prod[:], h1s[:], w2r_sb[:])
                red = small.tile([C, 1], f32)
                nc.vector.reduce_sum(red[:], prod[:], axis=mybir.AxisListType.X)
                cf = small.tile([C, 1], f32)
                nc.vector.tensor_add(cf[:], red[:], b2r_sb[:])
                # coef padded to 128 partitions for a K=128 matmul (small-K
                # matmuls are flaky on HW).
                coef128 = small.tile([128, 1], f32)
                nc.vector.memset(coef128[:], 0.0)
                nc.vector.scalar_tensor_tensor(
                    out=coef128[0:C, :], in0=cf[:], scalar=NEG, in1=cf[:],
                    op0=Alu.mult, op1=Alu.max,
                )
                with tc.tile_pool(name="psum_u", bufs=1, space="PSUM") as pup:
                    u_ps = pup.tile([128, 1], f32)
                    # u128[p] = coef[p%32] * d[p%32] / 32  (t128 folds d/32)
                    nc.tensor.matmul(
                        u_ps[:], lhsT=t128_sb[:], rhs=coef128[:],
                        start=True, stop=True,
                    )
                    nc.vector.tensor_scalar_mul(S_sb[:], m0_sb[:], u_ps[:])

            # ---- weighted channel sum ----
            # 8 groups per batch: matmul j scatters its 4 rows into
            # partitions 4j..4j+3 of one [32, 512] PSUM bank (via the m0big
            # column masking), so row m of the bank is out row 32t + m.
            with (
                tc.tile_pool(name="psum_o", bufs=4, space="PSUM") as pop,
                tc.tile_pool(name="stg", bufs=3) as stgp,
            ):
                for t in range(16):
                    po = pop.tile([32, W], f32)
                    for j in range(8):
                        g = 8 * t + j
                        nc.tensor.matmul(
                            po[:],
                            lhsT=S_sb[:, C * j : C * (j + 1)],
                            rhs=xs_sb[:, W * g : W * (g + 1)],
                            start=(j == 0), stop=(j == 7),
                        )
                    stg = stgp.tile([32, W], f32)
                    if t % 2 == 0:
                        nc.scalar.copy(stg[:], po[:])
                    else:
                        nc.vector.tensor_copy(stg[:], po[:])
                    nc.sync.dma_start(out_t[32 * t : 32 * (t + 1), :], stg[:])

    nc.compile()
    return nc


def _get_program():
    if "nc" not in _prog_cache:
        _prog_cache["nc"] = _build_program()
    return _prog_cache["nc"]


def prep_in_maps(x, dist, W1, b1, W2, b2):
    bf16 = ml_dtypes.bfloat16
    x = np.asarray(x, dtype=np.float32)
    dist = np.asarray(dist, dtype=np.float32)
    W1 = np.asarray(W1, dtype=np.float32)
    b1 = np.asarray(b1, dtype=np.float32)
    W2 = np.asarray(W2, dtype=np.float32)
    b2 = np.asarray(b2, dtype=np.float32)

    d = dist / dist.sum()
    # A[(w*32+c), n] = d[c]/16 * delta(c, n): folds w-sum + avgpool norm + d
    amat = np.tile(np.diag(d / (P * P)).astype(np.float32), (P, 1)).astype(bf16)
    # m0big[(wl*32+c), 36*j + wl] = 1 (see kernel: 8-group output batches)
    m0 = np.zeros((128, 8 * C), np.float32)
    for j in range(8):
        for wl in range(P):
            m0[wl * 32 : wl * 32 + C, 36 * j + wl] = 1.0
    m0 = m0.astype(bf16)
    # T128[c, p] = d[c]/32 * delta(c, p % 32), zero-padded to K=128 rows
    t128 = np.zeros((128, 128), np.float32)
    t128[0:C] = np.tile(np.diag(d / C).astype(np.float32), (1, P))
    b1r = np.ascontiguousarray(np.broadcast_to(b1, (C, HID))).astype(np.float32)
    w2r = np.ascontiguousarray(np.broadcast_to(W2[0], (C, HID))).astype(np.float32)
    b2r = np.full((C, 1), b2[0], dtype=np.float32)

    # x layout permutation: [b, c, k, (g4 j wl), h] -> [b, g4, (wl c), (j k h)]
    # so each quad-of-groups DMA is [128, 4096] with 16 KiB contiguous per
    # partition (no arithmetic, just a copy).
    xr = np.ascontiguousarray(
        x.reshape(B, C, K, G // 4, 4, P, W).transpose(0, 3, 5, 1, 4, 2, 6)
    ).reshape(B, G // 4, 128, 4 * K * W)

    # W1.T reordered in quads of row-chunks: w1q[gq, p, i*HID + col]
    # = W1.T[512*gq + 128*i + p, col]
    w1q = np.ascontiguousarray(
        W1.T.astype(bf16).reshape(G // 4, 4, 128, HID).transpose(0, 2, 1, 3)
    ).reshape(G // 4, 128, 4 * HID)

    return [
        dict(
            x=xr[b],
            w1q=w1q,
            amat=amat,
            m0=m0,
            t128=t128,
            b1r=b1r,
            w2r=w2r,
            b2r=b2r,
        )
        for b in range(N_CORES)
    ]


def kernel(x, dist, W1, b1, W2, b2):
    from concourse.bass_utils import run_bass_kernel_spmd

    in_maps = prep_in_maps(x, dist, W1, b1, W2, b2)
    nc = _get_program()
    res = run_bass_kernel_spmd(nc, in_maps, list(range(N_CORES)))
    out = np.stack([res.results[i]["out"] for i in range(N_CORES)])
    return out[:, None, :, :].astype(np.float32)
